# revision 1
# baseline (speedup 1.0000x reference)
"""Trainium2 Bass kernel for a pre-LN transformer block (B=2, S=2048, H=2048,
NH=32, HD=64, FFN=8192), run SPMD on 8 NeuronCores.

Sharding: data-parallel over batch (2 groups of 4 cores) x sequence-parallel
within the group (512 query tokens per core). Each core recomputes LN1 and
K/V for its whole batch element (no collectives), computes Q/attention/
proj/MLP for its own 512 tokens, and writes its [512, 2048] output slice.

All activations are kept transposed ([feature, token]) so every GEMM maps
onto nc.tensor.matmul(lhsT, rhs) directly; matmuls run in float32r (KQV,
scores, proj, w1) and bf16 (probs@V, w2). LN gains/biases and all linear
biases are folded into weights/residuals on the host.

Per-core token permutation trick: the host rotates each core's own 512
tokens to columns 0:512 of xT, so the SPMD program always takes Q from
columns 0:512. Attention is permutation-equivariant over keys, so K/V
ordering is irrelevant.
"""
import sys

sys.path.insert(0, '/opt/trn_rl_repo')

from contextlib import ExitStack

import numpy as np

import concourse.bacc as bacc
from concourse import masks, mybir, tile
from concourse.bass_utils import run_bass_kernel_spmd

F32 = mybir.dt.float32
F32R = mybir.dt.float32r
BF16 = mybir.dt.bfloat16
AF = mybir.ActivationFunctionType

B, S, H, NH, HD, FFN = 2, 2048, 2048, 32, 64, 8192
EPS = 1e-5
NCORE = 8
GRP = 4                   # cores per batch element
Q = S // GRP              # 512 query tokens per core
HT = H // 128             # 16 h-tiles
FT = FFN // 128           # 64 ffn tiles
TQ = S // 512             # 4 token chunks of 512
QT = Q // 128             # 4 q-tiles of 128


def _emit(nc, tc):
    # ---------------- DRAM parameters ----------------
    xT = nc.declare_dram_parameter("xT", [H, S], F32R, isOutput=False)
    xq = nc.declare_dram_parameter("xq", [Q, H], F32, isOutput=False)
    wq = nc.declare_dram_parameter("wq", [H, H], F32R, isOutput=False)
    wk = nc.declare_dram_parameter("wk", [H, H], F32R, isOutput=False)
    wv = nc.declare_dram_parameter("wv", [H, H], F32R, isOutput=False)
    wp = nc.declare_dram_parameter("wp", [H, H], F32R, isOutput=False)
    w1 = nc.declare_dram_parameter("w1", [H, FFN], F32R, isOutput=False)
    w2 = nc.declare_dram_parameter("w2", [FFN, H], BF16, isOutput=False)
    bqc = nc.declare_dram_parameter("bqc", [128, HT], F32, isOutput=False)
    bkc = nc.declare_dram_parameter("bkc", [128, HT], F32, isOutput=False)
    b1c = nc.declare_dram_parameter("b1c", [128, FT], F32, isOutput=False)
    b2r = nc.declare_dram_parameter("b2r", [1, H], F32R, isOutput=False)
    out = nc.declare_dram_parameter("out", [Q, H], F32, isOutput=True)

    P = lambda **kw: tc.alloc_tile_pool(**kw)

    const = P(name="const", bufs=1)
    dramp = P(name="dramp", bufs=1, space="DRAM")
    kt_dram = dramp.tile([H, S], F32R, tag="kt_dram", name="kt_dram")
    v_dram = dramp.tile([S, NH * 65], BF16, tag="v_dram", name="v_dram")

    # constants
    onef = const.tile([1, 128], F32, tag="onef", name="onef")
    nc.gpsimd.memset(onef[:], 1.0)
    onecol = const.tile([1, 128], F32R, tag="onecol", name="onecol")
    nc.vector.tensor_copy(onecol[:], onef[:])
    onesumf = const.tile([128, 1], F32, tag="onesumf", name="onesumf")
    nc.gpsimd.memset(onesumf[:], 1.0)
    onesum = const.tile([128, 1], F32R, tag="onesum", name="onesum")
    nc.vector.tensor_copy(onesum[:], onesumf[:])
    eps_sb = const.tile([128, 1], F32, tag="eps_sb", name="eps_sb")
    nc.gpsimd.memset(eps_sb[:], EPS)
    ones_bf32 = const.tile([128, NH], BF16, tag="ones_bf32", name="ones_bf32")
    nc.gpsimd.memset(ones_bf32[:], 1.0)
    ident = const.tile([128, 128], F32, tag="ident", name="ident")
    masks.make_identity(nc, ident[:])
    bq_sb = const.tile([128, HT], F32, tag="bq_sb", name="bq_sb")
    nc.sync.dma_start(bq_sb[:], bqc[:, :])
    bk_sb = const.tile([128, HT], F32, tag="bk_sb", name="bk_sb")
    nc.sync.dma_start(bk_sb[:], bkc[:, :])
    b1_sb = const.tile([128, FT], F32, tag="b1_sb", name="b1_sb")
    nc.sync.dma_start(b1_sb[:], b1c[:, :])
    b2_row = const.tile([1, H], F32R, tag="b2_row", name="b2_row")
    nc.sync.dma_start(b2_row[:], b2r[:, :])

    # ================= Phase 1: LN1 -> h1T =================
    h1p = P(name="h1p", bufs=1)
    h1T = [h1p.tile([128, S], F32R, tag=f"h1T{i}", name=f"h1T{i}") for i in range(HT)]

    ln1p = P(name="ln1p", bufs=1)
    xtp = P(name="xtp", bufs=2)
    sqp = P(name="sqp", bufs=1)
    ps_row = P(name="ps_row", bufs=1, space="PSUM")
    ps1 = [ps_row.tile([1, 512], F32, tag=f"s1_{t}", name=f"s1_{t}") for t in range(TQ)]
    ps2 = [ps_row.tile([1, 512], F32, tag=f"s2_{t}", name=f"s2_{t}") for t in range(TQ)]
    for ht in range(HT):
        xt = xtp.tile([128, S], F32R, tag="xt", name="xt")
        nc.sync.dma_start(xt[:], xT[ht * 128:(ht + 1) * 128, :])
        sq = sqp.tile([128, S], F32R, tag="sq", name="sq")
        nc.scalar.square(sq[:], xt[:])
        for t in range(TQ):
            nc.tensor.matmul(ps1[t][:], onesum[:], xt[:, t * 512:(t + 1) * 512],
                             start=(ht == 0), stop=(ht == HT - 1))
            nc.tensor.matmul(ps2[t][:], onesum[:], sq[:, t * 512:(t + 1) * 512],
                             start=(ht == 0), stop=(ht == HT - 1))
    rs_b = ln1p.tile([128, S], F32R, tag="rs_b", name="rs_b")
    murs_b = ln1p.tile([128, S], F32R, tag="murs_b", name="murs_b")
    rowp = P(name="rowp", bufs=1)
    rs_rows, murs_rows = [], []
    for t in range(TQ):
        mu = rowp.tile([1, 512], F32, tag="mu", name="mu")
        e2 = rowp.tile([1, 512], F32, tag="e2", name="e2")
        nc.scalar.mul(mu[:], ps1[t][:], 1.0 / H)
        nc.scalar.mul(e2[:], ps2[t][:], 1.0 / H)
        var = rowp.tile([1, 512], F32, tag="var", name="var")
        nc.vector.tensor_mul(var[:], mu[:], mu[:])
        nc.vector.tensor_sub(var[:], e2[:], var[:])
        std = rowp.tile([1, 512], F32, tag="std", name="std")
        nc.scalar.activation(std[:], var[:], AF.Sqrt, bias=eps_sb[0:1, :])
        rs = rowp.tile([1, 512], F32R, tag=f"rs{t}", name=f"rs{t}")
        nc.vector.reciprocal(rs[:], std[:])
        murs = rowp.tile([1, 512], F32R, tag=f"murs{t}", name=f"murs{t}")
        nc.vector.tensor_mul(murs[:], mu[:], rs[:])
        rs_rows.append(rs)
        murs_rows.append(murs)
    ps_row.release()
    ps_bc = P(name="ps_bc", bufs=2, space="PSUM")
    for t in range(TQ):
        sl = slice(t * 512, (t + 1) * 512)
        pb = ps_bc.tile([128, 512], F32, tag="pb", name="pb")
        nc.tensor.matmul(pb[:], onecol[:], rs_rows[t][:], start=True, stop=True)
        nc.vector.tensor_copy(rs_b[:, sl], pb[:])
        pb2 = ps_bc.tile([128, 512], F32, tag="pb2", name="pb2")
        nc.tensor.matmul(pb2[:], onecol[:], murs_rows[t][:], start=True, stop=True)
        nc.vector.tensor_copy(murs_b[:, sl], pb2[:])
    rowp.release()

    for ht in range(HT):
        xt = xtp.tile([128, S], F32R, tag="xt", name="xt")
        nc.sync.dma_start(xt[:], xT[ht * 128:(ht + 1) * 128, :])
        nc.vector.tensor_mul(h1T[ht][:], xt[:], rs_b[:])
        nc.vector.tensor_sub(h1T[ht][:], h1T[ht][:], murs_b[:])

    sqp.release()
    xtp.release()
    ln1p.release()
    ps_bc.release()

    # ================= Phase 2: KT, V, QT =================
    stg = P(name="stg", bufs=4)
    wvp = P(name="wvp", bufs=2)
    wkp = P(name="wkp", bufs=2)
    ps_mm = P(name="ps_mm", bufs=4, space="PSUM")
    for ft in range(HT):
        wt = wkp.tile([128, H], F32R, tag="wt", name="wt")
        nc.sync.dma_start(
            wt[:].rearrange("p (ht f) -> p ht f", ht=HT),
            wk.rearrange("(ht p) f -> p ht f", p=128)[:, :, ft * 128:(ft + 1) * 128])
        for t in range(TQ):
            pm = ps_mm.tile([128, 512], F32, tag="pm", name="pm")
            for ht in range(HT):
                nc.tensor.matmul(pm[:], wt[:, ht * 128:(ht + 1) * 128],
                                 h1T[ht][:, t * 512:(t + 1) * 512],
                                 start=(ht == 0), stop=(ht == HT - 1))
            st = stg.tile([128, 512], F32R, tag="st", name="st")
            nc.vector.tensor_scalar_add(st[:], pm[:], bk_sb[:, ft:ft + 1])
            nc.sync.dma_start(kt_dram[ft * 128:(ft + 1) * 128, t * 512:(t + 1) * 512], st[:])
    wkp.release()

    # --- V (256-col chunks to fit SBUF at full f32r rate) ---
    for fc in range(8):
        wt = wvp.tile([128, HT * 256], F32R, tag="wtv", name="wtv")
        nc.sync.dma_start(
            wt[:].rearrange("p (ht f) -> p ht f", ht=HT),
            wv.rearrange("(ht p) f -> p ht f", p=128)[:, :, fc * 256:(fc + 1) * 256])
        for tt in range(HT):
            pm = ps_mm.tile([128, 256], F32, tag="pmv", name="pmv")
            for ht in range(HT):
                nc.tensor.matmul(pm[:], h1T[ht][:, tt * 128:(tt + 1) * 128],
                                 wt[:, ht * 256:(ht + 1) * 256],
                                 start=(ht == 0), stop=(ht == HT - 1))
            st = stg.tile([128, 256], BF16, tag="stv", name="stv")
            nc.vector.tensor_copy(st[:], pm[:])
            vslice = v_dram[tt * 128:(tt + 1) * 128, :].rearrange(
                "p (h d) -> p h d", d=65)[:, 4 * fc:4 * fc + 4, 0:64]
            nc.sync.dma_start(vslice, st[:].rearrange("p (h d) -> p h d", d=64))
    for tt in range(HT):
        ocol = v_dram[tt * 128:(tt + 1) * 128, :].rearrange(
            "p (h d) -> p h d", d=65)[:, :, 64:65]
        nc.sync.dma_start(ocol, ones_bf32[:].rearrange("p (h d) -> p h d", d=1))
    wvp.release()

    # --- QT (own tokens are columns 0:Q of h1T) ---
    wqp = P(name="wqp", bufs=2)
    qtp = P(name="qtp", bufs=1, side="right")
    qt_sb = [qtp.tile([128, Q], F32R, tag=f"qt{i}", name=f"qt{i}") for i in range(HT)]
    for ft in range(HT):
        wt = wqp.tile([128, H], F32R, tag="wtq", name="wtq")
        nc.sync.dma_start(
            wt[:].rearrange("p (ht f) -> p ht f", ht=HT),
            wq.rearrange("(ht p) f -> p ht f", p=128)[:, :, ft * 128:(ft + 1) * 128])
        pm = ps_mm.tile([128, 512], F32, tag="pm", name="pm")
        for ht in range(HT):
            nc.tensor.matmul(pm[:], wt[:, ht * 128:(ht + 1) * 128], h1T[ht][:, 0:Q],
                             start=(ht == 0), stop=(ht == HT - 1))
        nc.vector.tensor_scalar_add(qt_sb[ft][:], pm[:], bq_sb[:, ft:ft + 1])
    wqp.release()
    stg.release()
    ps_mm.release()
    h1p.release()

    # ================= Phase 3: attention =================
    ctxp = P(name="ctxp", bufs=1)
    ctxT = [ctxp.tile([128, Q], F32R, tag=f"ctxT{i}", name=f"ctxT{i}") for i in range(HT)]

    kthp = P(name="kthp", bufs=2)
    vhp = P(name="vhp", bufs=2)
    expp = P(name="expp", bufs=3)
    att_sm = P(name="att_sm", bufs=2)
    ps_s = P(name="ps_s", bufs=2, space="PSUM")
    ps_c = P(name="ps_c", bufs=1, space="PSUM")
    ps_r = P(name="ps_r", bufs=1, space="PSUM")
    for hp in range(NH // 2):          # head pairs (2*hp, 2*hp+1)
        kth = kthp.tile([128, S], F32R, tag="kth", name="kth")
        nc.sync.dma_start(kth[:], kt_dram[hp * 128:(hp + 1) * 128, :])
        qt_pair = qt_sb[hp]
        vh = []
        pcs = []
        for j in range(2):
            h = 2 * hp + j
            v = vhp.tile([128, HT * 65], BF16, tag=f"vh{j}", name=f"vh{j}")
            nc.sync.dma_start(
                v[:].rearrange("p (tt d) -> p tt d", tt=HT),
                v_dram.rearrange("(tt p) f -> p tt f", p=128)[:, :, h * 65:(h + 1) * 65])
            vh.append(v)
            pcs.append(ps_c.tile([65, 512], F32, tag=f"pc{j}", name=f"pc{j}"))
        for kt in range(HT):
            pscr = ps_s.tile([128, 1024], F32, tag="pscr", name="pscr")
            for j in range(2):
                nc.tensor.matmul(pscr[:, j * 512:(j + 1) * 512],
                                 kth[j * 64:(j + 1) * 64, kt * 128:(kt + 1) * 128],
                                 qt_pair[j * 64:(j + 1) * 64, :], start=True, stop=True)
            ex = expp.tile([128, 1024], BF16, tag="ex", name="ex")
            nc.scalar.activation(ex[:], pscr[:], AF.Exp, scale=0.125)
            for j in range(2):
                nc.tensor.matmul(pcs[j][:], vh[j][:, kt * 65:(kt + 1) * 65],
                                 ex[:, j * 512:(j + 1) * 512],
                                 start=(kt == 0), stop=(kt == HT - 1))
        for j in range(2):
            rcp = att_sm.tile([1, 512], F32R, tag=f"rcp{j}", name=f"rcp{j}")
            nc.vector.reciprocal(rcp[:], pcs[j][64:65, :])
            pr = ps_r.tile([64, 512], F32, tag="pr", name="pr")
            nc.tensor.matmul(pr[:], onecol[:, 0:64], rcp[:], start=True, stop=True)
            rb = att_sm.tile([64, 512], F32, tag=f"rb{j}", name=f"rb{j}")
            nc.vector.tensor_copy(rb[:], pr[:])
            dst = ctxT[hp][j * 64:(j + 1) * 64, :]
            nc.vector.tensor_mul(dst, pcs[j][0:64, :], rb[:])
    att_sm.release()
    expp.release()
    vhp.release()
    kthp.release()
    ps_r.release()
    ps_c.release()
    ps_s.release()
    qtp.release()

    # ================= Phase 4: proj + residual + LN2 + transpose =================
    x2p = P(name="x2p", bufs=1, side="right")
    x2_sb = [x2p.tile([128, H], F32, tag=f"x2{i}", name=f"x2{i}") for i in range(QT)]
    ln2p = P(name="ln2p", bufs=1, side="right")
    mu2 = ln2p.tile([128, QT], F32, tag="mu2", name="mu2")
    s2c = ln2p.tile([128, QT], F32, tag="s2c", name="s2c")

    wpp = P(name="wpp", bufs=2)
    xqp_ = P(name="xqp_", bufs=2)
    ps_p = P(name="ps_p", bufs=4, space="PSUM")
    for fc in range(8):
        wt = wpp.tile([128, HT * 256], F32R, tag="wtp", name="wtp")
        nc.sync.dma_start(
            wt[:].rearrange("p (ht f) -> p ht f", ht=HT),
            wp.rearrange("(ht p) f -> p ht f", p=128)[:, :, fc * 256:(fc + 1) * 256])
        for qt in range(QT):
            pm = ps_p.tile([128, 256], F32, tag="pmp", name="pmp")
            for ht in range(HT):
                nc.tensor.matmul(pm[:], ctxT[ht][:, qt * 128:(qt + 1) * 128],
                                 wt[:, ht * 256:(ht + 1) * 256],
                                 start=(ht == 0), stop=(ht == HT - 1))
            xqt = xqp_.tile([128, 256], F32, tag="xqt", name="xqt")
            nc.sync.dma_start(xqt[:], xq[qt * 128:(qt + 1) * 128, fc * 256:(fc + 1) * 256])
            xsl = x2_sb[qt][:, fc * 256:(fc + 1) * 256]
            nc.vector.tensor_add(xsl, pm[:], xqt[:])
            ps1_ = xqp_.tile([128, 1], F32, tag="ps1_", name="ps1_")
            nc.vector.reduce_sum(ps1_[:], xsl, axis=mybir.AxisListType.X)
            sq_ = xqp_.tile([128, 256], F32, tag="sq_", name="sq_")
            ps2_ = xqp_.tile([128, 1], F32, tag="ps2_", name="ps2_")
            nc.scalar.activation(sq_[:], xsl, AF.Square, accum_out=ps2_[:])
            if fc == 0:
                nc.vector.tensor_copy(mu2[:, qt:qt + 1], ps1_[:])
                nc.vector.tensor_copy(s2c[:, qt:qt + 1], ps2_[:])
            else:
                nc.vector.tensor_add(mu2[:, qt:qt + 1], mu2[:, qt:qt + 1], ps1_[:])
                nc.vector.tensor_add(s2c[:, qt:qt + 1], s2c[:, qt:qt + 1], ps2_[:])
    xqp_.release()
    wpp.release()
    ps_p.release()
    ctxp.release()

    # incremental stats were accumulated during proj; finish per-qt rows
    nc.vector.tensor_scalar_mul(mu2[:], mu2[:], 1.0 / H)
    nc.vector.tensor_scalar_mul(s2c[:], s2c[:], 1.0 / H)
    var2 = ln2p.tile([128, QT], F32, tag="var2", name="var2")
    nc.vector.tensor_mul(var2[:], mu2[:], mu2[:])
    nc.vector.tensor_sub(var2[:], s2c[:], var2[:])
    std2 = ln2p.tile([128, QT], F32, tag="std2", name="std2")
    nc.scalar.activation(std2[:], var2[:], AF.Sqrt, bias=eps_sb[:, :])
    rs2 = ln2p.tile([128, QT], F32, tag="rs2", name="rs2")
    nc.vector.reciprocal(rs2[:], std2[:])

    h2tp = P(name="h2tp", bufs=1)
    h2T = [h2tp.tile([128, Q], F32R, tag=f"h2T{i}", name=f"h2T{i}") for i in range(HT)]
    h2n = P(name="h2n", bufs=4)
    ps_t = P(name="ps_t", bufs=4, space="PSUM")
    for qt in range(QT):
        for ht in range(HT):
            h2c = h2n.tile([128, 128], F32, tag="h2c", name="h2c")
            nc.vector.tensor_scalar(h2c[:], x2_sb[qt][:, ht * 128:(ht + 1) * 128],
                                    mu2[:, qt:qt + 1], rs2[:, qt:qt + 1],
                                    op0=mybir.AluOpType.subtract, op1=mybir.AluOpType.mult)
            pt = ps_t.tile([128, 128], F32, tag="pt", name="pt")
            nc.tensor.transpose(pt[:], h2c[:], ident[:])
            nc.vector.tensor_copy(h2T[ht][:, qt * 128:(qt + 1) * 128], pt[:])
    h2n.release()
    ps_t.release()
    ln2p.release()

    # ================= Phase 5: MLP up (y1T -> gelu -> g1T) =================
    g1p = P(name="g1p", bufs=1, side="right")
    g1T = [g1p.tile([128, Q], BF16, tag=f"g1T{i}", name=f"g1T{i}") for i in range(FT)]
    w1p = P(name="w1p", bufs=2)
    ps_y1 = P(name="ps_y1", bufs=4, space="PSUM")
    for Ft in range(FT):
        wt = w1p.tile([128, H], F32R, tag="w1t", name="w1t")
        nc.sync.dma_start(
            wt[:].rearrange("p (ht f) -> p ht f", ht=HT),
            w1.rearrange("(ht p) f -> p ht f", p=128)[:, :, Ft * 128:(Ft + 1) * 128])
        pm = ps_y1.tile([128, 512], F32, tag="pm1", name="pm1")
        for ht in range(HT):
            nc.tensor.matmul(pm[:], wt[:, ht * 128:(ht + 1) * 128], h2T[ht][:],
                             start=(ht == 0), stop=(ht == HT - 1))
        nc.scalar.activation(g1T[Ft][:], pm[:], AF.Gelu_apprx_tanh, bias=b1_sb[:, Ft:Ft + 1])
    w1p.release()
    ps_y1.release()
    h2tp.release()

    # ================= Phase 6: MLP down + residual + out =================
    w2p = P(name="w2p", bufs=3)
    finp = P(name="finp", bufs=2)
    b2bp = P(name="b2bp", bufs=1)
    b2_b = b2bp.tile([128, H], F32, tag="b2_b", name="b2_b")
    ps_b2 = P(name="ps_b2", bufs=2, space="PSUM")
    for t in range(TQ):
        pb = ps_b2.tile([128, 512], F32, tag="pb2b", name="pb2b")
        nc.tensor.matmul(pb[:], onecol[:], b2_row[:, t * 512:(t + 1) * 512],
                         start=True, stop=True)
        nc.vector.tensor_copy(b2_b[:, t * 512:(t + 1) * 512], pb[:])
    ps_b2.release()
    ps_y2 = P(name="ps_y2", bufs=1, space="PSUM")
    for half in range(2):
        pms = [[ps_y2.tile([128, 512], F32, tag=f"py_{qt}_{fc}", name=f"py_{qt}_{fc}")
                for fc in range(2)] for qt in range(QT)]
        for Ft in range(FT):
            wt = w2p.tile([128, 1024], BF16, tag="w2t", name="w2t")
            nc.sync.dma_start(wt[:], w2[Ft * 128:(Ft + 1) * 128,
                                        half * 1024:(half + 1) * 1024])
            for qt in range(QT):
                for fc in range(2):
                    nc.tensor.matmul(pms[qt][fc][:],
                                     g1T[Ft][:, qt * 128:(qt + 1) * 128],
                                     wt[:, fc * 512:(fc + 1) * 512],
                                     start=(Ft == 0), stop=(Ft == FT - 1))
        for qt in range(QT):
            for fc in range(2):
                col = half * 1024 + fc * 512
                fin = finp.tile([128, 512], F32, tag="fin", name="fin")
                nc.vector.tensor_add(fin[:], pms[qt][fc][:], x2_sb[qt][:, col:col + 512])
                nc.vector.tensor_add(fin[:], fin[:], b2_b[:, col:col + 512])
                nc.sync.dma_start(out[qt * 128:(qt + 1) * 128, col:col + 512], fin[:])
    b2bp.release()
    finp.release()
    w2p.release()
    ps_y2.release()
    g1p.release()
    x2p.release()
    const.release()
    dramp.release()


def _build():
    nc = bacc.Bacc(None, target_bir_lowering=False, debug=False)
    with tile.TileContext(nc, pool_alloc_mode="queue") as tc:
        with nc.allow_low_precision(reason="f32r-typed tiles share f32 bits; matmul accumulation stays fp32"):
            _emit(nc, tc)
    nc.compile()
    return nc


def _prep(inputs):
    x = np.asarray(inputs["x"], dtype=np.float32)
    ln1_g = np.asarray(inputs["ln1_g"], np.float32)
    ln1_b = np.asarray(inputs["ln1_b"], np.float32)
    w_qkv = np.asarray(inputs["w_qkv"], np.float32)
    b_qkv = np.asarray(inputs["b_qkv"], np.float32)
    w_proj = np.asarray(inputs["w_proj"], np.float32)
    b_proj = np.asarray(inputs["b_proj"], np.float32)
    ln2_g = np.asarray(inputs["ln2_g"], np.float32)
    ln2_b = np.asarray(inputs["ln2_b"], np.float32)
    w1_ = np.asarray(inputs["w1"], np.float32)
    b1_ = np.asarray(inputs["b1"], np.float32)
    w2_ = np.asarray(inputs["w2"], np.float32)
    b2_ = np.asarray(inputs["b2"], np.float32)

    wq3 = w_qkv.reshape(H, NH, 3, HD)
    w_q = np.ascontiguousarray(wq3[:, :, 0, :].reshape(H, H))
    w_k = np.ascontiguousarray(wq3[:, :, 1, :].reshape(H, H))
    w_v = np.ascontiguousarray(wq3[:, :, 2, :].reshape(H, H))
    b3 = b_qkv.reshape(NH, 3, HD)
    b_q, b_k, b_v = (b3[:, i, :].reshape(H) for i in range(3))

    wq_s = w_q * ln1_g[:, None]
    wk_s = w_k * ln1_g[:, None]
    wv_s = w_v * ln1_g[:, None]
    bq_f = b_q + ln1_b @ w_q
    bk_f = b_k + ln1_b @ w_k
    bv_f = b_v + ln1_b @ w_v
    bproj_f = b_proj + bv_f @ w_proj
    w1_s = w1_ * ln2_g[:, None]
    b1_f = b1_ + ln2_b @ w1_

    bq_cols = np.ascontiguousarray(bq_f.reshape(HT, 128).T)
    bk_cols = np.ascontiguousarray(bk_f.reshape(HT, 128).T)
    b1_cols = np.ascontiguousarray(b1_f.reshape(FT, 128).T)
    b2_row = np.ascontiguousarray(b2_.reshape(1, H))

    import ml_dtypes
    w2_bf = w2_.astype(ml_dtypes.bfloat16)
    shared = dict(wq=wq_s, wk=wk_s, wv=wv_s, wp=np.ascontiguousarray(w_proj),
                  w1=w1_s, w2=w2_bf, bqc=bq_cols, bkc=bk_cols, b1c=b1_cols,
                  b2r=b2_row)

    in_maps = []
    for c in range(NCORE):
        b, chunk = divmod(c, GRP)
        q0 = chunk * Q
        xb = x[b]
        perm = np.concatenate([np.arange(q0, q0 + Q), np.arange(0, q0),
                               np.arange(q0 + Q, S)])
        xT = np.ascontiguousarray(xb[perm].T)
        xq = xb[q0:q0 + Q] + bproj_f[None, :]
        m = dict(shared)
        m["xT"] = xT
        m["xq"] = np.ascontiguousarray(xq)
        in_maps.append(m)
    return in_maps


_CACHE = {}


def _get_exec():
    """Build + compile once; return (sharded_jit, meta) for repeat calls."""
    if 'exec' in _CACHE:
        return _CACHE['exec']
    import jax
    from jax.sharding import Mesh, PartitionSpec
    from jax.experimental.shard_map import shard_map
    from concourse import bass2jax, mybir as _mybir

    bass2jax.install_neuronx_cc_hook()
    nc = _build()

    partition_name = nc.partition_id_tensor.name if nc.partition_id_tensor else None
    in_names, out_names, out_avals = [], [], []
    for alloc in nc.m.functions[0].allocations:
        if not isinstance(alloc, _mybir.MemoryLocationSet):
            continue
        name = alloc.memorylocations[0].name
        if alloc.kind == "ExternalInput":
            if name != partition_name:
                in_names.append(name)
        elif alloc.kind == "ExternalOutput":
            shape = tuple(alloc.tensor_shape)
            dtype = _mybir.dt.np(alloc.dtype)
            out_names.append(name)
            out_avals.append(jax.core.ShapedArray(shape, dtype))
    n_params = len(in_names)
    all_in_names = in_names + out_names
    if partition_name is not None:
        all_in_names = all_in_names + [partition_name]

    def _body(*args):
        operands = list(args)
        if partition_name is not None:
            operands.append(bass2jax.partition_id_tensor())
        outs = bass2jax._bass_exec_p.bind(
            *operands,
            out_avals=tuple(out_avals),
            in_names=tuple(all_in_names),
            out_names=tuple(out_names),
            lowering_input_output_aliases=(),
            sim_require_finite=True,
            sim_require_nnan=True,
            nc=nc,
        )
        return tuple(outs)

    devices = jax.devices()[:NCORE]
    mesh = Mesh(np.asarray(devices), ("core",))
    n_outs = len(out_names)
    sharded = jax.jit(
        shard_map(_body, mesh=mesh,
                  in_specs=(PartitionSpec("core"),) * (n_params + n_outs),
                  out_specs=(PartitionSpec("core"),) * n_outs,
                  check_rep=False),
        keep_unused=True,
    )
    meta = dict(in_names=in_names, out_names=out_names, out_avals=out_avals,
                mesh=mesh, nc=nc)
    _CACHE['exec'] = (sharded, meta)
    return _CACHE['exec']


def _device_inputs(inputs):
    """Concat per-core inputs on axis 0 and put on the 8 devices."""
    import jax
    from jax.sharding import NamedSharding, PartitionSpec
    sharded, meta = _get_exec()
    in_maps = _prep(inputs)
    concat = []
    for name in meta['in_names']:
        arrs = [in_maps[c][name] for c in range(NCORE)]
        concat.append(np.concatenate(arrs, axis=0))
    for av in meta['out_avals']:
        concat.append(np.zeros((NCORE * av.shape[0],) + tuple(av.shape[1:]), av.dtype))
    sh = NamedSharding(meta['mesh'], PartitionSpec("core"))
    return [jax.device_put(a, sh) for a in concat]


def _execute(dev_args):
    import jax
    sharded, meta = _get_exec()
    outs = sharded(*dev_args)
    jax.block_until_ready(outs)
    return outs


def _assemble(outs, meta):
    arr = np.asarray(outs[0]).reshape(NCORE, Q, H)
    full = np.empty((B, S, H), np.float32)
    for c in range(NCORE):
        b, chunk = divmod(c, GRP)
        full[b, chunk * Q:(chunk + 1) * Q] = arr[c]
    return full


def _run(inputs, trace=False, trace_kwargs=None):
    sharded, meta = _get_exec()
    dev_args = _device_inputs(inputs)
    outs = _execute(dev_args)
    return _assemble(outs, meta), None


def kernel(**inputs):
    out, _ = _run(inputs)
    return out

